# revision 1
# baseline (speedup 1.0000x reference)
"""AttentionBlock (GroupNorm + single-head self-attention + residual) on 8 TRN2 cores.

Strategy: pure data-parallel over batch (16 items -> 2 per core), no collectives.
Per batch item, on one core (c=512 channels, n=1024 positions, 32 groups):
  - GroupNorm: per-channel bn_stats over n, group-combine via a block-diagonal
    selector matmul (groups are 16 consecutive channels), apply as per-partition
    scale/shift fused into one tensor_scalar -> hn (bf16).
  - Q, K: matmul(lhsT=w{q,k}^T blocks, rhs=hn) -> [c, n] layout, + bias.
  - V computed TRANSPOSED: matmul(lhsT=hn blocks, rhs=wv^T) -> vT [n, c] layout.
  - Attention computed TRANSPOSED: S^T = matmul(lhsT=k, rhs=q) -> [j, i] tiles;
    exp (no max-subtraction: logits are O(+-5), safe in f32 psum) fused into the
    PSUM eviction on the scalar engine with the 1/sqrt(c) scale folded in.
  - Softmax denominators: ones-vector matmul over the j (partition) axis.
  - out = V @ attn: matmul(lhsT=vT blocks, rhs=exp tiles) -> [c, i] unnormalized.
  - proj: matmul(lhsT=wp^T blocks, rhs=out); eviction multiplies by the
    broadcast reciprocal denominators (normalization commutes with the
    projection over the i axis), adds bias' = wp@bv + bp (the V bias also
    commutes: sum_j attn[i,j] = 1), adds the residual x, -> f32 out.
All matmul operands bf16 (1 PE cycle/row vs fp32's 4), f32 accumulation.

The two batch items are software-pipelined: batch 1's loads + GroupNorm are
emitted in the middle of batch 0's attention phases so the in-order engines
(DVE/ACT/gpsimd) overlap them with batch 0's matmuls and the PE never sees a
multi-microsecond idle window (which would re-throttle the HAM clock gate).
PSUM eviction ops work on [128, 1024] (two banks) to halve per-op overhead.
"""

import numpy as np
import ml_dtypes

B_TOT, C, H, W = 16, 512, 32, 32
N = H * W            # 1024
NCORES = 8
BPC = B_TOT // NCORES  # 2 batch items per core
CT = C // 128        # 4 channel tiles
NT = N // 128        # 8 position tiles
NCH = N // 512       # 2 free-dim chunks of 512
GS = 16              # group size (channels per group)
EPS = 1e-5
SCALE = float(C) ** -0.5

_CACHE = {}


def _build_bass():
    import concourse.bass as bass  # noqa: F401
    import concourse.tile as tile
    from concourse import bacc, mybir

    F32 = mybir.dt.float32
    BF16 = mybir.dt.bfloat16
    Alu = mybir.AluOpType
    Act = mybir.ActivationFunctionType

    nc = bacc.Bacc("TRN2", target_bir_lowering=False, debug=False,
                   num_devices=NCORES)

    x_ext = nc.dram_tensor("x", [BPC, 128, CT, N], F32, kind="ExternalInput").ap()
    w_ext = {
        name: nc.dram_tensor(name, [128, CT, 512], BF16, kind="ExternalInput").ap()
        for name in ("wq", "wk", "wv", "wp")
    }
    vec_ext = {
        name: nc.dram_tensor(name, [128, CT], F32, kind="ExternalInput").ap()
        for name in ("gamma", "beta", "bq", "bk", "bpp")
    }
    sel_ext = nc.dram_tensor("sel", [128, 128], F32, kind="ExternalInput").ap()
    ones_ext = nc.dram_tensor("ones", [128, 1], BF16, kind="ExternalInput").ap()
    out_ext = nc.dram_tensor("out", [BPC, 128, CT, N], F32, kind="ExternalOutput").ap()

    with tile.TileContext(nc) as tc:
        with (
            tc.tile_pool(name="consts", bufs=1) as consts,
            tc.tile_pool(name="xp", bufs=2) as xp,
            tc.tile_pool(name="hnp", bufs=2) as hnp,
            tc.tile_pool(name="qkp", bufs=1) as qkp,
            tc.tile_pool(name="vp", bufs=1) as vp,
            tc.tile_pool(name="ep", bufs=1) as ep,
            tc.tile_pool(name="oup", bufs=1) as oup,
            tc.tile_pool(name="outp", bufs=3) as outp,
            tc.tile_pool(name="xbp", bufs=2) as xbp,
            tc.tile_pool(name="rp", bufs=1) as rp,
            tc.tile_pool(name="smallp", bufs=8) as smallp,
            tc.tile_pool(name="psq", bufs=2, space="PSUM") as psq,
            tc.tile_pool(name="psv", bufs=2, space="PSUM") as psv,
            tc.tile_pool(name="psg", bufs=1, space="PSUM") as psg,
        ):
            # ---- constants ----
            # Small consts go FIRST on the sync queue (they gate the first
            # GroupNorm matmul); x tiles follow on sync; the four 0.5MB
            # weight tensors split across the scalar/gpsimd trigger queues.
            vec_sb = {}
            for name in ("gamma", "beta", "bq", "bk", "bpp"):
                vec_sb[name] = consts.tile([128, CT], F32, tag=name,
                                           name=f"vec_{name}")
                nc.sync.dma_start(vec_sb[name][:], vec_ext[name][:])
            sel_sb = consts.tile([128, 128], F32, tag="sel")
            nc.sync.dma_start(sel_sb[:], sel_ext[:])
            ones_sb = consts.tile([128, 1], BF16, tag="ones")
            nc.sync.dma_start(ones_sb[:], ones_ext[:])
            w_sb = {}
            for i, name in enumerate(("wq", "wk", "wv", "wp")):
                w_sb[name] = consts.tile([128, CT, 512], BF16, tag=name,
                                         name=f"w_{name}")
                eng = nc.scalar if i < 2 else nc.gpsimd
                eng.dma_start(w_sb[name][:], w_ext[name][:])
            magic_sb = consts.tile([128, 1], mybir.dt.int32, tag="magic")
            nc.vector.memset(magic_sb[:], 0x5F3759DF)

            # PE warm-up: throwaway matmuls fill the initial DMA wait so the
            # HAM clock gate is already released (2.4 GHz) when the real
            # matmuls start (a >3.4us PE idle window re-throttles it).
            wu_sb = consts.tile([128, 512], BF16, tag="wu")
            nc.vector.memset(wu_sb[:], 0.0)
            ps_wu = psv.tile([128, 512], F32, tag="vmm", name="ps_warm")
            for i in range(24):
                nc.tensor.matmul(ps_wu[:], wu_sb[:, 0:128], wu_sb[:],
                                 start=(i == 0), stop=(i == 23))
            nc.vector.tensor_copy(wu_sb[:, 0:4], ps_wu[:, 0:4])

            def load_x(b):
                # spread the four 512KB tile loads over three trigger queues
                engs = [nc.sync, nc.scalar, nc.gpsimd, nc.sync]
                xts = []
                for t in range(CT):
                    xt = xp.tile([128, N], F32, tag=f"x{t}", name=f"x_b{b}_t{t}")
                    engs[t].dma_start(xt[:], x_ext[b, :, t, :])
                    xts.append(xt)
                return xts

            def gn(b, xts):
                # per-channel stats for all 4 channel tiles, then ONE
                # group-combine matmul and ONE 4-wide rsqrt chain
                mv = smallp.tile([128, CT, 2], F32, tag="mv", name=f"mv{b}")
                for t in range(CT):
                    stats = smallp.tile([128, 2, 6], F32, tag="stats",
                                        name=f"st{b}_{t}")
                    nc.vector.bn_stats(stats[:, 0, :], xts[t][:, 0:512])
                    nc.vector.bn_stats(stats[:, 1, :], xts[t][:, 512:1024])
                    nc.vector.bn_aggr(mv[:, t, :], stats[:])
                # s_all[:, 0, t]=mean_t, s_all[:, 1, t]=E[x^2]_t
                s_all = smallp.tile([128, 2, CT], F32, tag="s_all", name=f"s{b}")
                nc.vector.tensor_copy(s_all[:, 0, :], mv[:, :, 0])
                nc.vector.tensor_tensor(s_all[:, 1, :], mv[:, :, 0], mv[:, :, 0],
                                        Alu.mult)
                nc.vector.tensor_tensor(s_all[:, 1, :], s_all[:, 1, :],
                                        mv[:, :, 1], Alu.add)
                gs = psg.tile([128, 2, CT], F32, tag="gs", name=f"gs{b}")
                nc.tensor.matmul(gs[:], sel_sb[:], s_all[:], start=True, stop=True)
                gsb = smallp.tile([128, 2, CT], F32, tag="gsb", name=f"gb{b}")
                nc.vector.tensor_copy(gsb[:], gs[:])
                ab = smallp.tile([128, 4, CT], F32, tag="ab", name=f"ab{b}")
                va = ab[:, 0, :]         # var
                vp_ = ab[:, 1, :]        # var + eps
                y = ab[:, 2, :]
                tmp = ab[:, 3, :]
                nc.vector.tensor_tensor(va, gsb[:, 0, :], gsb[:, 0, :], Alu.mult)
                nc.vector.tensor_tensor(va, gsb[:, 1, :], va, Alu.subtract)
                # rstd = rsqrt(var+eps) entirely on DVE (fast-inverse-sqrt seed
                # + 2 Newton steps) so the scalar engine's activation tables
                # never leave the exp set (table reloads are 2.7us each).
                nc.vector.tensor_scalar_add(vp_, va, EPS)
                I32 = mybir.dt.int32
                nc.vector.tensor_scalar(y.bitcast(I32), vp_.bitcast(I32), 1,
                                        None, Alu.arith_shift_right)
                nc.vector.tensor_tensor(y.bitcast(I32),
                                        magic_sb[:].to_broadcast([128, CT]),
                                        y.bitcast(I32), Alu.subtract)
                for _ in range(2):  # Newton: y *= 1.5 - 0.5*v*y^2
                    nc.vector.tensor_tensor(tmp, y, y, Alu.mult)
                    nc.vector.tensor_tensor(tmp, tmp, vp_, Alu.mult)
                    nc.vector.tensor_scalar(tmp, tmp, -0.5, 1.5, Alu.mult,
                                            Alu.add)
                    nc.vector.tensor_tensor(y, y, tmp, Alu.mult)
                a_all = ab[:, 0, :]      # reuse var slot: a = rstd*gamma
                bsh = ab[:, 3, :]
                nc.vector.tensor_tensor(a_all, y, vec_sb["gamma"][:], Alu.mult)
                nc.vector.tensor_tensor(bsh, gsb[:, 0, :], a_all, Alu.mult)
                nc.vector.tensor_tensor(bsh, vec_sb["beta"][:], bsh, Alu.subtract)
                hn_sb = hnp.tile([128, CT, N], BF16, tag="hn", name=f"hn{b}")
                for t in range(CT):
                    nc.vector.tensor_scalar(hn_sb[:, t, :], xts[t][:],
                                            ab[:, 0, t:t + 1], ab[:, 3, t:t + 1],
                                            Alu.mult, Alu.add)
                return hn_sb

            def qkv(b, hn_sb):
                q_sb = qkp.tile([128, CT, N], BF16, tag="q", name=f"q{b}")
                k_sb = qkp.tile([128, CT, N], BF16, tag="k", name=f"k{b}")
                for t in range(CT):
                    ps = psq.tile([128, N], F32, tag="mm", name=f"psq{b}_{t}")
                    for ch in range(NCH):
                        cs = slice(ch * 512, (ch + 1) * 512)
                        for it in range(CT):
                            nc.tensor.matmul(
                                ps[:, cs], w_sb["wq"][:, it, t * 128:(t + 1) * 128],
                                hn_sb[:, it, cs],
                                start=(it == 0), stop=(it == CT - 1))
                    nc.scalar.activation(q_sb[:, t, :], ps[:], Act.Identity,
                                         bias=vec_sb["bq"][:, t:t + 1])
                    ps2 = psq.tile([128, N], F32, tag="mm", name=f"psk{b}_{t}")
                    for ch in range(NCH):
                        cs = slice(ch * 512, (ch + 1) * 512)
                        for it in range(CT):
                            nc.tensor.matmul(
                                ps2[:, cs], w_sb["wk"][:, it, t * 128:(t + 1) * 128],
                                hn_sb[:, it, cs],
                                start=(it == 0), stop=(it == CT - 1))
                    nc.scalar.activation(k_sb[:, t, :], ps2[:], Act.Identity,
                                         bias=vec_sb["bk"][:, t:t + 1])
                # V, transposed: vT[n, c] (no bias; folded into bpp)
                vT_sb = vp.tile([128, NT, 512], BF16, tag="vT", name=f"vT{b}")
                for jt in range(NT):
                    ps = psv.tile([128, 512], F32, tag="vmm", name=f"psv{b}_{jt}")
                    for it in range(CT):
                        nc.tensor.matmul(
                            ps[:], hn_sb[:, it, jt * 128:(jt + 1) * 128],
                            w_sb["wv"][:, it, :],
                            start=(it == 0), stop=(it == CT - 1))
                    nc.vector.tensor_copy(vT_sb[:, jt, :], ps[:])
                return q_sb, k_sb, vT_sb

            def st_exp(b, q_sb, k_sb):
                e_sb = ep.tile([128, NT, N], BF16, tag="e", name=f"e{b}")
                for jt in range(NT):
                    ps = psq.tile([128, N], F32, tag="mm", name=f"pss{b}_{jt}")
                    for ch in range(NCH):
                        cs = slice(ch * 512, (ch + 1) * 512)
                        for ct in range(CT):
                            nc.tensor.matmul(
                                ps[:, cs], k_sb[:, ct, jt * 128:(jt + 1) * 128],
                                q_sb[:, ct, cs],
                                start=(ct == 0), stop=(ct == CT - 1))
                    nc.scalar.activation(e_sb[:, jt, :], ps[:], Act.Exp,
                                         scale=SCALE)
                return e_sb

            def sums_recip(b, e_sb):
                # pre-reduce the 8 exp tiles elementwise on DVE so the
                # partition-sum needs only 2 ones-matmuls instead of 16
                e_sum = rp.tile([128, N], BF16, tag="esum", name=f"es{b}")
                nc.vector.tensor_copy(e_sum[:], e_sb[:, 0, :])
                for jt in range(1, NT):
                    nc.vector.tensor_tensor(e_sum[:], e_sum[:], e_sb[:, jt, :],
                                            Alu.add)
                sums_sb = rp.tile([1, N], F32, tag="sums", name=f"sm{b}")
                for ch in range(NCH):
                    cs = slice(ch * 512, (ch + 1) * 512)
                    ps1 = psg.tile([1, 512], F32, tag="onesum", name=f"os{b}_{ch}")
                    nc.tensor.matmul(ps1[:], ones_sb[:], e_sum[:, cs],
                                     start=True, stop=True)
                    nc.vector.tensor_copy(sums_sb[:, cs], ps1[:])
                sumb_sb = rp.tile([128, N], F32, tag="sumb", name=f"sb{b}")
                nc.gpsimd.partition_broadcast(sumb_sb[:], sums_sb[:])
                recip_sb = rp.tile([128, N], F32, tag="recip", name=f"rc{b}")
                nc.vector.reciprocal_approx_fast(recip_sb[:], sumb_sb[:])
                return recip_sb

            def pv(b, vT_sb, e_sb):
                ou_sb = oup.tile([128, CT, N], BF16, tag="ou", name=f"ou{b}")
                for ct in range(CT):
                    ps = psq.tile([128, N], F32, tag="mm", name=f"pso{b}_{ct}")
                    for ch in range(NCH):
                        cs = slice(ch * 512, (ch + 1) * 512)
                        for jt in range(NT):
                            nc.tensor.matmul(
                                ps[:, cs], vT_sb[:, jt, ct * 128:(ct + 1) * 128],
                                e_sb[:, jt, cs],
                                start=(jt == 0), stop=(jt == NT - 1))
                    nc.scalar.copy(ou_sb[:, ct, :], ps[:])
                return ou_sb

            def make_xb(b, xts):
                # xb = x + bias' precomputed off the critical path on gpsimd,
                # so the proj eviction is just (psum*recip) + xb on DVE
                xb = xbp.tile([128, CT, N], F32, tag="xb", name=f"xb{b}")
                for t in range(CT):
                    nc.gpsimd.tensor_scalar_add(xb[:, t, :], xts[t][:],
                                                vec_sb["bpp"][:, t:t + 1])
                return xb

            def proj(b, ou_sb, recip_sb, xb):
                for ot in range(CT):
                    ps = psq.tile([128, N], F32, tag="mm", name=f"psp{b}_{ot}")
                    for ch in range(NCH):
                        cs = slice(ch * 512, (ch + 1) * 512)
                        for ct in range(CT):
                            nc.tensor.matmul(
                                ps[:, cs], w_sb["wp"][:, ct, ot * 128:(ot + 1) * 128],
                                ou_sb[:, ct, cs],
                                start=(ct == 0), stop=(ct == CT - 1))
                    o_sb = outp.tile([128, N], F32, tag="o", name=f"o{b}_{ot}")
                    nc.vector.tensor_tensor(o_sb[:], ps[:], recip_sb[:], Alu.mult)
                    nc.vector.tensor_tensor(o_sb[:], o_sb[:], xb[:, ot, :],
                                            Alu.add)
                    nc.sync.dma_start(out_ext[b, :, ot, :], o_sb[:])

            # ---- software pipeline over the two batch items ----
            x0 = load_x(0)
            h0 = gn(0, x0)
            xb0 = make_xb(0, x0)
            q0, k0, v0 = qkv(0, h0)
            x1 = load_x(1)
            e0 = st_exp(0, q0, k0)
            h1 = gn(1, x1)
            xb1 = make_xb(1, x1)
            r0 = sums_recip(0, e0)
            o0 = pv(0, v0, e0)
            q1, k1, v1 = qkv(1, h1)
            proj(0, o0, r0, xb0)
            e1 = st_exp(1, q1, k1)
            r1 = sums_recip(1, e1)
            o1 = pv(1, v1, e1)
            proj(1, o1, r1, xb1)

    nc.compile()
    return nc


def _prep_vec(v):
    # [C] f32 -> [128, CT] with v_sb[p, t] = v[t*128 + p]
    return np.ascontiguousarray(
        np.asarray(v, dtype=np.float32).reshape(CT, 128).T)


def _prep_w(w):
    # [C, C] (out, in) -> lhsT layout [128, CT, 512] bf16:
    # w_sb[p, it, o] = w.T[it*128 + p, o] = w[o, it*128 + p]
    wT = np.asarray(w, dtype=np.float32).T
    return np.ascontiguousarray(
        wT.reshape(CT, 128, C).transpose(1, 0, 2).astype(ml_dtypes.bfloat16))


def kernel(x, gamma, beta, wq, bq, wk, bk, wv, bv, wp, bp):
    from concourse.bass_utils import run_bass_kernel_spmd

    nc = _CACHE.get("nc")
    if nc is None:
        nc = _CACHE["nc"] = _build_bass()

    x = np.asarray(x, dtype=np.float32)
    # [16, C, H, W] -> [16, 128, CT, N]
    xr = np.ascontiguousarray(
        x.reshape(B_TOT, CT, 128, N).transpose(0, 2, 1, 3))

    bpp = np.asarray(wp, np.float32) @ np.asarray(bv, np.float32) \
        + np.asarray(bp, np.float32)
    sel = np.kron(np.eye(128 // GS, dtype=np.float32),
                  np.full((GS, GS), 1.0 / GS, dtype=np.float32))
    common = {
        "wq": _prep_w(wq), "wk": _prep_w(wk), "wv": _prep_w(wv),
        "wp": _prep_w(wp),
        "gamma": _prep_vec(gamma), "beta": _prep_vec(beta),
        "bq": _prep_vec(bq), "bk": _prep_vec(bk), "bpp": _prep_vec(bpp),
        "sel": sel, "ones": np.ones((128, 1), dtype=ml_dtypes.bfloat16),
    }
    in_maps = [
        {"x": np.ascontiguousarray(xr[c * BPC:(c + 1) * BPC]), **common}
        for c in range(NCORES)
    ]
    res = run_bass_kernel_spmd(nc, in_maps, core_ids=list(range(NCORES)))
    # [BPC, 128, CT, N] per core -> [16, C, H, W]
    out = np.concatenate([r["out"] for r in res.results], axis=0)
    return np.ascontiguousarray(
        out.transpose(0, 2, 1, 3)).reshape(B_TOT, C, H, W)



# revision 5
# speedup vs baseline: 2.2984x; 2.2984x over previous
"""AttentionBlock (GroupNorm + single-head self-attention + residual) on 8 TRN2 cores.

Strategy: pure data-parallel over batch (16 items -> 2 per core), no collectives.
All six big matmuls per item (Q, K, V, S=K^T Q, PV, proj) run in fp8-e4m3 with
perf_mode=DoubleRow (2 contraction sub-tiles per pass -> ~2x PE throughput).
Weights are pre-scaled by 64 on the host (w ~ N(0, 1/c) would underflow fp8's
normal range); the 1/64 is folded into the PSUM evictions.  x is shipped as
bf16 (GroupNorm stats + residual tolerate it at rel-err 4e-3 << the 2e-2 gate).

Per item (c=512 channels, n=1024 positions, 32 groups of 16 channels):
  - GroupNorm: bn_stats per channel-tile, group-combine via block-diagonal
    selector matmul, rsqrt on DVE (fast-inverse-sqrt + 2 Newton steps so the
    scalar engine's activation tables never swap), apply -> hn fp8.
  - Q,K: DoubleRow matmuls (lhsT = pre-transposed fp8 weights); Q evicted on
    DVE (tensor_scalar mult 1/64 add bias), K on ACT -- engine balance.
  - V computed TRANSPOSED: matmul(lhsT=hn, rhs=wv) -> vT [n, c], DVE eviction.
  - S^T = K^T Q -> [j, i] tiles; eviction on ACT: e = exp(S*scale - 3) in fp8.
    The -3 shift keeps e <= ~120 < fp8e4's 240 ceiling (the fp8 downcast is
    NONSAT: overflow would be Inf, not saturation); softmax cancels it exactly.
  - Denominators: bf16 running sum of e tiles on DVE, then ones(=1/8)-matmul
    to a [1, n] row, broadcast back to 128 partitions with a K=1 matmul
    (no gpsimd: its bulk ops run at ~7 GB/s and drag concurrent DVE ops
    into lockstep -- the previous version lost ~140us to exactly that),
    reciprocal_approx_fast -> recip carries the 8x ou pre-scale.
  - out = V @ e -> eviction multiplies by recip (DVE) -> ou fp8 (scaled 8x).
  - proj: DoubleRow matmul; eviction applies 1/(64*8), adds bpp = wp@bv + bp
    (V bias commutes through softmax) and the bf16 residual x.
The two items are software-pipelined with explicit interleaving: item 0's S
phase interleaves with item 1's QKV, item 0's proj with item 1's S, so the PE
never idles long enough to re-throttle the HAM clock gate, and the ACT/DVE
eviction load stays under the PE's production rate in every region.
"""

import numpy as np
import ml_dtypes

B_TOT, C, H, W = 16, 512, 32, 32
N = H * W            # 1024
NCORES = 8
BPC = B_TOT // NCORES  # 2 batch items per core
CT = C // 128        # 4 channel tiles
NT = N // 128        # 8 position tiles
NCH = N // 512       # 2 free-dim chunks of 512
GS = 16              # group size (channels per group)
EPS = 1e-5
SCALE = float(C) ** -0.5
WS = 64.0            # weight pre-scale (folded out at evictions)
OUS = 8.0            # ou pre-scale (folded into recip via ones=1/8)
EXPB = -3.0          # exp logit shift (cancels in softmax)

_CACHE = {}


def _build_bass():
    import concourse.bass as bass  # noqa: F401
    import concourse.tile as tile
    from concourse import bacc, mybir

    F32 = mybir.dt.float32
    BF16 = mybir.dt.bfloat16
    F8 = mybir.dt.float8e4
    Alu = mybir.AluOpType
    Act = mybir.ActivationFunctionType
    DR = mybir.MatmulPerfMode.DoubleRow

    nc = bacc.Bacc("TRN2", target_bir_lowering=False, debug=False,
                   num_devices=NCORES)

    x_ext = nc.dram_tensor("x", [BPC, 128, CT, N], BF16, kind="ExternalInput").ap()
    w_ext = {
        name: nc.dram_tensor(name, [128, CT, 512], F8, kind="ExternalInput").ap()
        for name in ("wq", "wk", "wv", "wp")
    }
    vec_ext = {
        name: nc.dram_tensor(name, [128, CT], F32, kind="ExternalInput").ap()
        for name in ("gamma", "beta", "bq", "bk", "bpp")
    }
    sel_ext = nc.dram_tensor("sel", [128, 128], F32, kind="ExternalInput").ap()
    ones_ext = nc.dram_tensor("ones", [128, 1], BF16, kind="ExternalInput").ap()
    onescol_ext = nc.dram_tensor("onescol", [1, 128], BF16, kind="ExternalInput").ap()
    out_ext = nc.dram_tensor("out", [BPC, 128, CT, N], F32, kind="ExternalOutput").ap()

    with tile.TileContext(nc) as tc:
        with (
            tc.tile_pool(name="consts", bufs=1) as consts,
            tc.tile_pool(name="xp", bufs=2) as xp,
            tc.tile_pool(name="hnp", bufs=2) as hnp,
            tc.tile_pool(name="qkp", bufs=2) as qkp,
            tc.tile_pool(name="vp", bufs=2) as vp,
            tc.tile_pool(name="ep", bufs=2) as ep,
            tc.tile_pool(name="oup", bufs=2) as oup,
            tc.tile_pool(name="outp", bufs=3) as outp,
            tc.tile_pool(name="rp", bufs=2) as rp,
            tc.tile_pool(name="smallp", bufs=8) as smallp,
            tc.tile_pool(name="psq", bufs=2, space="PSUM") as psq,
            tc.tile_pool(name="psv", bufs=2, space="PSUM") as psv,
            tc.tile_pool(name="pssm", bufs=2, space="PSUM") as pssm,
        ):
            # ---- constants ----
            # Small consts go FIRST on the sync queue (they gate the first
            # GroupNorm combine); x tiles follow; the four 0.25MB fp8 weight
            # tensors split across the scalar/gpsimd trigger queues.
            vec_sb = {}
            for name in ("gamma", "beta", "bq", "bk", "bpp"):
                vec_sb[name] = consts.tile([128, CT], F32, tag=name,
                                           name=f"vec_{name}")
                nc.sync.dma_start(vec_sb[name][:], vec_ext[name][:])
            sel_sb = consts.tile([128, 128], F32, tag="sel")
            nc.sync.dma_start(sel_sb[:], sel_ext[:])
            ones_sb = consts.tile([128, 1], BF16, tag="ones")
            nc.sync.dma_start(ones_sb[:], ones_ext[:])
            onescol_sb = consts.tile([1, 128], BF16, tag="onescol")
            nc.sync.dma_start(onescol_sb[:], onescol_ext[:])
            w_sb = {}
            for i, name in enumerate(("wq", "wk", "wv", "wp")):
                w_sb[name] = consts.tile([128, CT, 512], F8, tag=name,
                                         name=f"w_{name}")
                eng = nc.scalar if i < 2 else nc.gpsimd
                eng.dma_start(w_sb[name][:], w_ext[name][:])
            magic_sb = consts.tile([128, 1], mybir.dt.int32, tag="magic")
            nc.vector.memset(magic_sb[:], 0x5F3759DF)
            expb_sb = consts.tile([128, 1], F32, tag="expb")
            nc.vector.memset(expb_sb[:], EXPB)

            # PE warm-up: throwaway matmuls fill the initial DMA wait so the
            # HAM clock gate is already released (2.4 GHz) when the real
            # matmuls start (a >3.4us PE idle window re-throttles it).
            wu_sb = consts.tile([128, 512], BF16, tag="wu")
            nc.vector.memset(wu_sb[:], 0.0)
            ps_wu = psv.tile([128, 512], F32, tag="vmm", name="ps_warm")
            for i in range(24):
                nc.tensor.matmul(ps_wu[:], wu_sb[:, 0:128], wu_sb[:],
                                 start=(i == 0), stop=(i == 23))
            nc.vector.tensor_copy(wu_sb[:, 0:4], ps_wu[:, 0:4])

            def load_x(b):
                engs = [nc.sync, nc.scalar, nc.gpsimd, nc.sync]
                xts = []
                for t in range(CT):
                    xt = xp.tile([128, N], BF16, tag=f"x{t}", name=f"x_b{b}_t{t}")
                    engs[t].dma_start(xt[:], x_ext[b, :, t, :])
                    xts.append(xt)
                return xts

            def gn(b, xts):
                # per-channel stats for all 4 channel tiles, then ONE
                # group-combine matmul and ONE 4-wide rsqrt chain
                mv = smallp.tile([128, CT, 2], F32, tag="mv", name=f"mv{b}")
                for t in range(CT):
                    stats = smallp.tile([128, 2, 6], F32, tag="stats",
                                        name=f"st{b}_{t}")
                    nc.vector.bn_stats(stats[:, 0, :], xts[t][:, 0:512])
                    nc.vector.bn_stats(stats[:, 1, :], xts[t][:, 512:1024])
                    nc.vector.bn_aggr(mv[:, t, :], stats[:])
                # s_all[:, 0, t]=mean_t, s_all[:, 1, t]=E[x^2]_t
                s_all = smallp.tile([128, 2, CT], F32, tag="s_all", name=f"s{b}")
                nc.vector.tensor_copy(s_all[:, 0, :], mv[:, :, 0])
                nc.vector.tensor_tensor(s_all[:, 1, :], mv[:, :, 0], mv[:, :, 0],
                                        Alu.mult)
                nc.vector.tensor_tensor(s_all[:, 1, :], s_all[:, 1, :],
                                        mv[:, :, 1], Alu.add)
                gs = pssm.tile([128, 2, CT], F32, tag="sm", name=f"gs{b}")
                nc.tensor.matmul(gs[:], sel_sb[:], s_all[:], start=True, stop=True)
                gsb = smallp.tile([128, 2, CT], F32, tag="gsb", name=f"gb{b}")
                nc.vector.tensor_copy(gsb[:], gs[:])
                ab = smallp.tile([128, 4, CT], F32, tag="ab", name=f"ab{b}")
                va = ab[:, 0, :]         # var
                vp_ = ab[:, 1, :]        # var + eps
                y = ab[:, 2, :]
                tmp = ab[:, 3, :]
                nc.vector.tensor_tensor(va, gsb[:, 0, :], gsb[:, 0, :], Alu.mult)
                nc.vector.tensor_tensor(va, gsb[:, 1, :], va, Alu.subtract)
                # rstd = rsqrt(var+eps) entirely on DVE (fast-inverse-sqrt seed
                # + 2 Newton steps) so the scalar engine's activation tables
                # never leave the exp set (table reloads are 2.7us each).
                nc.vector.tensor_scalar_add(vp_, va, EPS)
                I32 = mybir.dt.int32
                nc.vector.tensor_scalar(y.bitcast(I32), vp_.bitcast(I32), 1,
                                        None, Alu.arith_shift_right)
                nc.vector.tensor_tensor(y.bitcast(I32),
                                        magic_sb[:].to_broadcast([128, CT]),
                                        y.bitcast(I32), Alu.subtract)
                for _ in range(2):  # Newton: y *= 1.5 - 0.5*v*y^2
                    nc.vector.tensor_tensor(tmp, y, y, Alu.mult)
                    nc.vector.tensor_tensor(tmp, tmp, vp_, Alu.mult)
                    nc.vector.tensor_scalar(tmp, tmp, -0.5, 1.5, Alu.mult,
                                            Alu.add)
                    nc.vector.tensor_tensor(y, y, tmp, Alu.mult)
                a_all = ab[:, 0, :]      # reuse var slot: a = rstd*gamma
                bsh = ab[:, 3, :]
                nc.vector.tensor_tensor(a_all, y, vec_sb["gamma"][:], Alu.mult)
                nc.vector.tensor_tensor(bsh, gsb[:, 0, :], a_all, Alu.mult)
                nc.vector.tensor_tensor(bsh, vec_sb["beta"][:], bsh, Alu.subtract)
                hn_sb = hnp.tile([128, CT, N], F8, tag="hn", name=f"hn{b}")
                for t in range(CT):
                    nc.vector.tensor_scalar(hn_sb[:, t, :], xts[t][:],
                                            ab[:, 0, t:t + 1], ab[:, 3, t:t + 1],
                                            Alu.mult, Alu.add)
                return hn_sb

            def qk_tile(b, hn_sb, dst, wname, bname, t, on_act):
                # dst[:, t, :] = psum/WS + bias, psum = w^T @ hn (DoubleRow)
                ps = psq.tile([128, N], F32, tag="mm", name=f"ps_{wname}{b}_{t}")
                for itp in range(2):
                    lhs = w_sb[wname][:, 2 * itp:2 * itp + 2, t * 128:(t + 1) * 128]
                    for ch in range(NCH):
                        cs = slice(ch * 512, (ch + 1) * 512)
                        nc.tensor.matmul(ps[:, cs], lhs,
                                         hn_sb[:, 2 * itp:2 * itp + 2, cs],
                                         start=(itp == 0), stop=(itp == 1),
                                         perf_mode=DR)
                bias = vec_sb[bname][:, t:t + 1]
                if on_act:
                    nc.scalar.activation(dst[:, t, :], ps[:], Act.Identity,
                                         bias=bias, scale=1.0 / WS)
                else:
                    nc.vector.tensor_scalar(dst[:, t, :], ps[:], 1.0 / WS,
                                            bias, Alu.mult, Alu.add)

            def v_tile(b, hn_sb, vT_sb, jt):
                # vT[:, jt, :] = (hn^T @ wv)/WS  (DoubleRow, transposed out)
                ps = psv.tile([128, 512], F32, tag="vmm", name=f"psv{b}_{jt}")
                for itp in range(2):
                    nc.tensor.matmul(
                        ps[:], hn_sb[:, 2 * itp:2 * itp + 2, jt * 128:(jt + 1) * 128],
                        w_sb["wv"][:, 2 * itp:2 * itp + 2, :],
                        start=(itp == 0), stop=(itp == 1), perf_mode=DR)
                nc.vector.tensor_scalar(vT_sb[:, jt, :], ps[:], 1.0 / WS,
                                        None, Alu.mult)

            def s_tile(b, q_sb, k_sb, e_sb, jt):
                # e[:, jt, :] = exp(scale * k[:, :, jt-tile]^T @ q + EXPB)
                ps = psq.tile([128, N], F32, tag="mm", name=f"pss{b}_{jt}")
                for ctp in range(2):
                    lhs = k_sb[:, 2 * ctp:2 * ctp + 2, jt * 128:(jt + 1) * 128]
                    for ch in range(NCH):
                        cs = slice(ch * 512, (ch + 1) * 512)
                        nc.tensor.matmul(ps[:, cs], lhs,
                                         q_sb[:, 2 * ctp:2 * ctp + 2, cs],
                                         start=(ctp == 0), stop=(ctp == 1),
                                         perf_mode=DR)
                # q,k both carry 1/WS already -> plain scale; EXPB cancels in
                # the softmax normalization.
                nc.scalar.activation(e_sb[:, jt, :], ps[:], Act.Exp,
                                     bias=expb_sb[:], scale=SCALE)

            def esum_add(b, esum_sb, e_sb, jt):
                if jt == 0:
                    nc.vector.tensor_copy(esum_sb[:], e_sb[:, 0, :])
                else:
                    nc.vector.tensor_tensor(esum_sb[:], esum_sb[:],
                                            e_sb[:, jt, :], Alu.add)

            def pv_with_sums(b, vT_sb, e_sb, esum_sb):
                # interleaves the softmax-denominator reduction (ones-matmul,
                # K=1 broadcast matmul, reciprocal) with the PV matmuls so the
                # PE never waits on the small DVE copies between them.
                drow = rp.tile([1, N], BF16, tag="drow", name=f"dr{b}")
                recip_sb = rp.tile([128, N], F32, tag="recip", name=f"rc{b}")
                ou_sb = oup.tile([128, CT, N], F8, tag="ou", name=f"ou{b}")
                pv_ps = []

                def pv_mms(ct):
                    ps = psq.tile([128, N], F32, tag="mm", name=f"pso{b}_{ct}")
                    for jtp in range(4):
                        lhs = vT_sb[:, 2 * jtp:2 * jtp + 2, ct * 128:(ct + 1) * 128]
                        for ch in range(NCH):
                            cs = slice(ch * 512, (ch + 1) * 512)
                            nc.tensor.matmul(ps[:, cs], lhs,
                                             e_sb[:, 2 * jtp:2 * jtp + 2, cs],
                                             start=(jtp == 0), stop=(jtp == 3),
                                             perf_mode=DR)
                    pv_ps.append(ps)

                def ou_evict(ct):
                    nc.vector.tensor_tensor(ou_sb[:, ct, :], pv_ps[ct][:],
                                            recip_sb[:], Alu.mult)

                psd0 = pssm.tile([1, 512], F32, tag="sm", name=f"d0{b}")
                nc.tensor.matmul(psd0[:], ones_sb[:], esum_sb[:, 0:512],
                                 start=True, stop=True)
                nc.vector.tensor_copy(drow[:, 0:512], psd0[:])
                pv_mms(0)
                psd1 = pssm.tile([1, 512], F32, tag="sm", name=f"d1{b}")
                nc.tensor.matmul(psd1[:], ones_sb[:], esum_sb[:, 512:1024],
                                 start=True, stop=True)
                nc.vector.tensor_copy(drow[:, 512:1024], psd1[:])
                pv_mms(1)
                bc0 = pssm.tile([128, 512], F32, tag="sm", name=f"bc0{b}")
                nc.tensor.matmul(bc0[:], onescol_sb[:], drow[:, 0:512],
                                 start=True, stop=True)
                nc.vector.reciprocal_approx_fast(recip_sb[:, 0:512], bc0[:])
                bc1 = pssm.tile([128, 512], F32, tag="sm", name=f"bc1{b}")
                nc.tensor.matmul(bc1[:], onescol_sb[:], drow[:, 512:1024],
                                 start=True, stop=True)
                nc.vector.reciprocal_approx_fast(recip_sb[:, 512:1024], bc1[:])
                pv_mms(2)
                ou_evict(0)
                ou_evict(1)
                pv_mms(3)
                ou_evict(2)
                ou_evict(3)
                return ou_sb

            def proj_tile(b, ou_sb, xts, ot, on_act):
                ps = psq.tile([128, N], F32, tag="mm", name=f"psp{b}_{ot}")
                for ctp in range(2):
                    lhs = w_sb["wp"][:, 2 * ctp:2 * ctp + 2, ot * 128:(ot + 1) * 128]
                    for ch in range(NCH):
                        cs = slice(ch * 512, (ch + 1) * 512)
                        nc.tensor.matmul(ps[:, cs], lhs,
                                         ou_sb[:, 2 * ctp:2 * ctp + 2, cs],
                                         start=(ctp == 0), stop=(ctp == 1),
                                         perf_mode=DR)
                o_sb = outp.tile([128, N], F32, tag="o", name=f"o{b}_{ot}")
                bias = vec_sb["bpp"][:, ot:ot + 1]
                if on_act:
                    nc.scalar.activation(o_sb[:], ps[:], Act.Identity,
                                         bias=bias, scale=1.0 / (WS * OUS))
                else:
                    nc.vector.tensor_scalar(o_sb[:], ps[:], 1.0 / (WS * OUS),
                                            bias, Alu.mult, Alu.add)
                nc.vector.tensor_tensor(o_sb[:], o_sb[:], xts[ot][:], Alu.add)
                nc.gpsimd.dma_start(out_ext[b, :, ot, :], o_sb[:])

            # ---- software pipeline over the two batch items ----
            x0 = load_x(0)
            x1 = load_x(1)
            h0 = gn(0, x0)

            q0 = qkp.tile([128, CT, N], F8, tag="q", name="q0")
            k0 = qkp.tile([128, CT, N], F8, tag="k", name="k0")
            v0 = vp.tile([128, NT, 512], F8, tag="vT", name="vT0")
            for t in range(CT):
                qk_tile(0, h0, q0, "wq", "bq", t, on_act=False)
                qk_tile(0, h0, k0, "wk", "bk", t, on_act=True)
                v_tile(0, h0, v0, 2 * t)
                v_tile(0, h0, v0, 2 * t + 1)

            h1 = gn(1, x1)

            # item0 S-phase interleaved with item1 QKV
            e0 = ep.tile([128, NT, N], F8, tag="e", name="e0")
            es0 = rp.tile([128, N], BF16, tag="esum", name="es0")
            q1 = qkp.tile([128, CT, N], F8, tag="q", name="q1")
            k1 = qkp.tile([128, CT, N], F8, tag="k", name="k1")
            v1 = vp.tile([128, NT, 512], F8, tag="vT", name="vT1")
            for r in range(CT):
                s_tile(0, q0, k0, e0, 2 * r)
                s_tile(0, q0, k0, e0, 2 * r + 1)
                esum_add(0, es0, e0, 2 * r)
                esum_add(0, es0, e0, 2 * r + 1)
                qk_tile(1, h1, q1, "wq", "bq", r, on_act=False)
                qk_tile(1, h1, k1, "wk", "bk", r, on_act=True)
                v_tile(1, h1, v1, 2 * r)
                v_tile(1, h1, v1, 2 * r + 1)

            ou0 = pv_with_sums(0, v0, e0, es0)

            # item0 proj interleaved with item1 S-phase
            e1 = ep.tile([128, NT, N], F8, tag="e", name="e1")
            es1 = rp.tile([128, N], BF16, tag="esum", name="es1")
            for r in range(CT):
                proj_tile(0, ou0, x0, r, on_act=False)
                s_tile(1, q1, k1, e1, 2 * r)
                s_tile(1, q1, k1, e1, 2 * r + 1)
                esum_add(1, es1, e1, 2 * r)
                esum_add(1, es1, e1, 2 * r + 1)

            ou1 = pv_with_sums(1, v1, e1, es1)
            for r in range(CT):
                proj_tile(1, ou1, x1, r, on_act=True)

    nc.compile()
    return nc


def _prep_vec(v):
    # [C] f32 -> [128, CT] with v_sb[p, t] = v[t*128 + p]
    return np.ascontiguousarray(
        np.asarray(v, dtype=np.float32).reshape(CT, 128).T)


def _prep_w(w):
    # [C, C] (out, in) -> lhsT layout [128, CT, 512] fp8e4, pre-scaled by WS:
    # w_sb[p, it, o] = w[o, it*128 + p] * WS
    wT = np.asarray(w, dtype=np.float32).T * WS
    arr = wT.reshape(CT, 128, C).transpose(1, 0, 2)
    return np.clip(np.ascontiguousarray(arr), -240.0, 240.0).astype(
        ml_dtypes.float8_e4m3)


def kernel(x, gamma, beta, wq, bq, wk, bk, wv, bv, wp, bp):
    from concourse.bass_utils import run_bass_kernel_spmd

    nc = _CACHE.get("nc")
    if nc is None:
        nc = _CACHE["nc"] = _build_bass()

    x = np.asarray(x, dtype=np.float32)
    # [16, C, H, W] -> [16, 128, CT, N] bf16
    xr = np.ascontiguousarray(
        x.reshape(B_TOT, CT, 128, N).transpose(0, 2, 1, 3)).astype(
        ml_dtypes.bfloat16)

    bpp = np.asarray(wp, np.float32) @ np.asarray(bv, np.float32) \
        + np.asarray(bp, np.float32)
    sel = np.kron(np.eye(128 // GS, dtype=np.float32),
                  np.full((GS, GS), 1.0 / GS, dtype=np.float32))
    common = {
        "wq": _prep_w(wq), "wk": _prep_w(wk), "wv": _prep_w(wv),
        "wp": _prep_w(wp),
        "gamma": _prep_vec(gamma), "beta": _prep_vec(beta),
        "bq": _prep_vec(bq), "bk": _prep_vec(bk), "bpp": _prep_vec(bpp),
        "sel": sel,
        "ones": np.full((128, 1), 1.0 / OUS, dtype=ml_dtypes.bfloat16),
        "onescol": np.ones((1, 128), dtype=ml_dtypes.bfloat16),
    }
    in_maps = [
        {"x": np.ascontiguousarray(xr[c * BPC:(c + 1) * BPC]), **common}
        for c in range(NCORES)
    ]
    res = run_bass_kernel_spmd(nc, in_maps, core_ids=list(range(NCORES)))
    # [BPC, 128, CT, N] per core -> [16, C, H, W]
    out = np.concatenate([r["out"] for r in res.results], axis=0)
    return np.ascontiguousarray(
        out.transpose(0, 2, 1, 3)).reshape(B_TOT, C, H, W)


# revision 7
# speedup vs baseline: 2.6136x; 1.1371x over previous
"""AttentionBlock (GroupNorm + single-head self-attention + residual) on 8 TRN2 cores.

Strategy: pure data-parallel over batch (16 items -> 2 per core), no collectives.
All six big matmuls per item (Q, K, V, S=K^T Q, PV, proj) run in fp8-e4m3 with
perf_mode=DoubleRow (2 contraction sub-tiles per pass -> 2x PE throughput).
Weights are pre-scaled by 64 on the host (w ~ N(0, 1/c) would underflow fp8's
normal range); the 1/64 is folded into the PSUM evictions.  x is shipped as
bf16 (GroupNorm stats + residual tolerate it at rel-err ~6e-3 << the 2e-2 gate).

Per item (c=512 channels, n=1024 positions, 32 groups of 16 channels):
  - GroupNorm: bn_stats per channel-tile, group-combine via block-diagonal
    selector matmul, rsqrt on DVE (fast-inverse-sqrt + 2 Newton steps so the
    scalar engine's activation tables never swap), apply -> hn fp8.
  - Q,K: DoubleRow matmuls, both evicted on ACT (scale 1/64 + bias) -- the
    QK stretch has no other ACT work, and this frees DVE for the *other*
    item's GroupNorm, which is emitted interleaved so hn(1) is ready the
    moment item 1's QKV matmuls come up.
  - V computed TRANSPOSED: matmul(lhsT=hn, rhs=wv) -> vT [n, c], DVE evict.
  - S^T = K^T Q -> [j, i] tiles; eviction on ACT: e = exp(S*scale - 3) fp8.
    The -3 shift keeps e <= ~120 < fp8e4's 240 ceiling (fp8 downcast is
    NONSAT: overflow would be Inf); softmax cancels the shift exactly.
  - Denominators WITHOUT any elementwise pass over e: 16 accumulating
    ones(=1/8)-matmuls reduce e over j into a [1, n] PSUM row right after
    the S phase (PE cost 3.4us, replaces a 9us serial DVE add-chain), then
    a K=1 matmul broadcasts the row to 128 partitions and
    reciprocal_approx_fast gives recip, which carries the 8x ou pre-scale.
  - out = V @ e -> eviction multiplies by recip (DVE) -> ou fp8 (scaled 8x).
  - proj: DoubleRow matmul; eviction applies 1/(64*8), adds bpp = wp@bv + bp
    (V bias commutes through softmax) and the bf16 residual x.
Startup: all small constants ride in ONE packed DMA (eight separate small
transfers previously delayed x's arrival by ~8us), x tiles and fp8 weights
spread over the sync/scalar/gpsimd trigger queues, and the PE runs ~40
throwaway matmuls so the HAM clock gate is released (2.4 GHz) before the
first real matmul.  Outputs fan out over four DMA queues so the tail isn't
serialized behind one queue.  The two items are software-pipelined: item 0's
S phase interleaves with item 1's QKV, item 0's proj with item 1's S.
"""

import numpy as np
import ml_dtypes

B_TOT, C, H, W = 16, 512, 32, 32
N = H * W            # 1024
NCORES = 8
BPC = B_TOT // NCORES  # 2 batch items per core
CT = C // 128        # 4 channel tiles
NT = N // 128        # 8 position tiles
NCH = N // 512       # 2 free-dim chunks of 512
GS = 16              # group size (channels per group)
EPS = 1e-5
SCALE = float(C) ** -0.5
WS = 64.0            # weight pre-scale (folded out at evictions)
OUS = 8.0            # ou pre-scale (folded into recip via ones=1/8)
EXPB = -3.0          # exp logit shift (cancels in softmax)
NVEC = 5             # gamma, beta, bq, bk, bpp
CB_W = NVEC * CT + 128  # const blob width (vectors + sel)

_CACHE = {}


def _build_bass():
    import concourse.bass as bass  # noqa: F401
    import concourse.tile as tile
    from concourse import bacc, mybir

    F32 = mybir.dt.float32
    BF16 = mybir.dt.bfloat16
    F8 = mybir.dt.float8e4
    Alu = mybir.AluOpType
    Act = mybir.ActivationFunctionType
    DR = mybir.MatmulPerfMode.DoubleRow

    nc = bacc.Bacc("TRN2", target_bir_lowering=False, debug=False,
                   num_devices=NCORES)

    x_ext = nc.dram_tensor("x", [BPC, 128, CT, N], BF16, kind="ExternalInput").ap()
    w_ext = {
        name: nc.dram_tensor(name, [128, CT, 512], F8, kind="ExternalInput").ap()
        for name in ("wq", "wk", "wv", "wp")
    }
    cb_ext = nc.dram_tensor("cb", [128, CB_W], F32, kind="ExternalInput").ap()
    out_ext = nc.dram_tensor("out", [BPC, 128, CT, N], F32, kind="ExternalOutput").ap()

    with tile.TileContext(nc) as tc:
        with (
            tc.tile_pool(name="consts", bufs=1) as consts,
            tc.tile_pool(name="xp", bufs=2) as xp,
            tc.tile_pool(name="hnp", bufs=2) as hnp,
            tc.tile_pool(name="qkp", bufs=2) as qkp,
            tc.tile_pool(name="vp", bufs=2) as vp,
            tc.tile_pool(name="ep", bufs=2) as ep,
            tc.tile_pool(name="oup", bufs=2) as oup,
            tc.tile_pool(name="outp", bufs=3) as outp,
            tc.tile_pool(name="rp", bufs=2) as rp,
            tc.tile_pool(name="smallp", bufs=8) as smallp,
            tc.tile_pool(name="psq", bufs=2, space="PSUM") as psq,
            tc.tile_pool(name="psv", bufs=2, space="PSUM") as psv,
            tc.tile_pool(name="pssm", bufs=2, space="PSUM") as pssm,
        ):
            # ---- constants: one packed DMA (gamma|beta|bq|bk|bpp|sel) ----
            cb_sb = consts.tile([128, CB_W], F32, tag="cb")
            nc.gpsimd.dma_start(cb_sb[:], cb_ext[:])
            vec_sb = {
                name: cb_sb[:, i * CT:(i + 1) * CT]
                for i, name in enumerate(("gamma", "beta", "bq", "bk", "bpp"))
            }
            sel_sb = cb_sb[:, NVEC * CT:NVEC * CT + 128]
            ones_sb = consts.tile([128, 1], BF16, tag="ones")
            nc.vector.memset(ones_sb[:], 1.0 / OUS)
            onescol_sb = consts.tile([1, 128], BF16, tag="onescol")
            nc.vector.memset(onescol_sb[:], 1.0)
            magic_sb = consts.tile([128, 1], mybir.dt.int32, tag="magic")
            nc.vector.memset(magic_sb[:], 0x5F3759DF)
            expb_sb = consts.tile([128, 1], F32, tag="expb")
            nc.vector.memset(expb_sb[:], EXPB)

            def load_x(b, engs):
                xts = []
                for t in range(CT):
                    xt = xp.tile([128, N], BF16, tag=f"x{t}", name=f"x_b{b}_t{t}")
                    engs[t].dma_start(xt[:], x_ext[b, :, t, :])
                    xts.append(xt)
                return xts

            # x first on every queue, then weights, then item 1's x
            x0 = load_x(0, [nc.sync, nc.scalar, nc.sync, nc.gpsimd])
            w_sb = {}
            for i, name in enumerate(("wq", "wk", "wv", "wp")):
                w_sb[name] = consts.tile([128, CT, 512], F8, tag=name,
                                         name=f"w_{name}")
                eng = nc.scalar if i < 2 else nc.gpsimd
                eng.dma_start(w_sb[name][:], w_ext[name][:])
            x1 = load_x(1, [nc.sync, nc.scalar, nc.sync, nc.gpsimd])

            # PE warm-up: throwaway matmuls fill the initial DMA + GroupNorm
            # wait so the HAM clock gate is already released (2.4 GHz) when
            # the real matmuls start (>3.4us PE idle re-throttles to 1.2).
            wu_sb = consts.tile([128, 512], BF16, tag="wu")
            nc.vector.memset(wu_sb[:], 0.0)
            ps_wu = psv.tile([128, 512], F32, tag="vmm", name="ps_warm")
            for i in range(40):
                nc.tensor.matmul(ps_wu[:], wu_sb[:, 0:128], wu_sb[:],
                                 start=(i == 0), stop=(i == 39))
            nc.vector.tensor_copy(wu_sb[:, 0:4], ps_wu[:, 0:4])

            def gn_stats(b, xts, mv, t):
                stats = smallp.tile([128, 2, 6], F32, tag="stats",
                                    name=f"st{b}_{t}")
                nc.vector.bn_stats(stats[:, 0, :], xts[t][:, 0:512])
                nc.vector.bn_stats(stats[:, 1, :], xts[t][:, 512:1024])
                nc.vector.bn_aggr(mv[:, t, :], stats[:])

            def gn_tail(b, xts, mv):
                # s_all[:, 0, t]=mean_t, s_all[:, 1, t]=E[x^2]_t
                s_all = smallp.tile([128, 2, CT], F32, tag="s_all", name=f"s{b}")
                nc.vector.tensor_copy(s_all[:, 0, :], mv[:, :, 0])
                nc.vector.tensor_tensor(s_all[:, 1, :], mv[:, :, 0], mv[:, :, 0],
                                        Alu.mult)
                nc.vector.tensor_tensor(s_all[:, 1, :], s_all[:, 1, :],
                                        mv[:, :, 1], Alu.add)
                gs = pssm.tile([128, 2, CT], F32, tag="sm", name=f"gs{b}")
                nc.tensor.matmul(gs[:], sel_sb, s_all[:], start=True, stop=True)
                gsb = smallp.tile([128, 2, CT], F32, tag="gsb", name=f"gb{b}")
                nc.vector.tensor_copy(gsb[:], gs[:])
                ab = smallp.tile([128, 4, CT], F32, tag="ab", name=f"ab{b}")
                va = ab[:, 0, :]         # var
                vp_ = ab[:, 1, :]        # var + eps
                y = ab[:, 2, :]
                tmp = ab[:, 3, :]
                nc.vector.tensor_tensor(va, gsb[:, 0, :], gsb[:, 0, :], Alu.mult)
                nc.vector.tensor_tensor(va, gsb[:, 1, :], va, Alu.subtract)
                # rstd = rsqrt(var+eps) entirely on DVE (fast-inverse-sqrt seed
                # + 2 Newton steps) so the scalar engine's activation tables
                # never leave the exp set (table reloads are 2.7us each).
                nc.vector.tensor_scalar_add(vp_, va, EPS)
                I32 = mybir.dt.int32
                nc.vector.tensor_scalar(y.bitcast(I32), vp_.bitcast(I32), 1,
                                        None, Alu.arith_shift_right)
                nc.vector.tensor_tensor(y.bitcast(I32),
                                        magic_sb[:].to_broadcast([128, CT]),
                                        y.bitcast(I32), Alu.subtract)
                for _ in range(2):  # Newton: y *= 1.5 - 0.5*v*y^2
                    nc.vector.tensor_tensor(tmp, y, y, Alu.mult)
                    nc.vector.tensor_tensor(tmp, tmp, vp_, Alu.mult)
                    nc.vector.tensor_scalar(tmp, tmp, -0.5, 1.5, Alu.mult,
                                            Alu.add)
                    nc.vector.tensor_tensor(y, y, tmp, Alu.mult)
                a_all = ab[:, 0, :]      # reuse var slot: a = rstd*gamma
                bsh = ab[:, 3, :]
                nc.vector.tensor_tensor(a_all, y, vec_sb["gamma"], Alu.mult)
                nc.vector.tensor_tensor(bsh, gsb[:, 0, :], a_all, Alu.mult)
                nc.vector.tensor_tensor(bsh, vec_sb["beta"], bsh, Alu.subtract)
                hn_sb = hnp.tile([128, CT, N], F8, tag="hn", name=f"hn{b}")
                for t in range(CT):
                    nc.vector.tensor_scalar(hn_sb[:, t, :], xts[t][:],
                                            ab[:, 0, t:t + 1], ab[:, 3, t:t + 1],
                                            Alu.mult, Alu.add)
                return hn_sb

            def gn_full(b, xts):
                mv = smallp.tile([128, CT, 2], F32, tag="mv", name=f"mv{b}")
                for t in range(CT):
                    gn_stats(b, xts, mv, t)
                return gn_tail(b, xts, mv)

            def qk_tile(b, hn_sb, dst, wname, bname, t):
                # dst[:, t, :] = psum/WS + bias, psum = w^T @ hn (DoubleRow)
                ps = psq.tile([128, N], F32, tag="mm", name=f"ps_{wname}{b}_{t}")
                for itp in range(2):
                    lhs = w_sb[wname][:, 2 * itp:2 * itp + 2, t * 128:(t + 1) * 128]
                    for ch in range(NCH):
                        cs = slice(ch * 512, (ch + 1) * 512)
                        nc.tensor.matmul(ps[:, cs], lhs,
                                         hn_sb[:, 2 * itp:2 * itp + 2, cs],
                                         start=(itp == 0), stop=(itp == 1),
                                         perf_mode=DR)
                nc.scalar.activation(dst[:, t, :], ps[:], Act.Identity,
                                     bias=vec_sb[bname][:, t:t + 1], scale=1.0 / WS)

            def v_tile(b, hn_sb, vT_sb, jt):
                # vT[:, jt, :] = (hn^T @ wv)/WS  (DoubleRow, transposed out)
                ps = psv.tile([128, 512], F32, tag="vmm", name=f"psv{b}_{jt}")
                for itp in range(2):
                    nc.tensor.matmul(
                        ps[:], hn_sb[:, 2 * itp:2 * itp + 2, jt * 128:(jt + 1) * 128],
                        w_sb["wv"][:, 2 * itp:2 * itp + 2, :],
                        start=(itp == 0), stop=(itp == 1), perf_mode=DR)
                nc.vector.tensor_scalar(vT_sb[:, jt, :], ps[:], 1.0 / WS,
                                        None, Alu.mult)

            def s_tile(b, q_sb, k_sb, e_sb, jt):
                # e[:, jt, :] = exp(scale * k[:, :, jt-tile]^T @ q + EXPB)
                ps = psq.tile([128, N], F32, tag="mm", name=f"pss{b}_{jt}")
                for ctp in range(2):
                    lhs = k_sb[:, 2 * ctp:2 * ctp + 2, jt * 128:(jt + 1) * 128]
                    for ch in range(NCH):
                        cs = slice(ch * 512, (ch + 1) * 512)
                        nc.tensor.matmul(ps[:, cs], lhs,
                                         q_sb[:, 2 * ctp:2 * ctp + 2, cs],
                                         start=(ctp == 0), stop=(ctp == 1),
                                         perf_mode=DR)
                nc.scalar.activation(e_sb[:, jt, :], ps[:], Act.Exp,
                                     bias=expb_sb[:], scale=SCALE)

            def dsum(b, e_sb):
                # D[i] = sum_j e[j, i] / 8 via 16 accumulating ones-matmuls
                # (no elementwise pass over e), then K=1 broadcast matmul and
                # approximate reciprocal -> recip = 8/D on all 128 partitions.
                drow = rp.tile([1, N], BF16, tag="drow", name=f"dr{b}")
                recip_sb = rp.tile([128, N], F32, tag="recip", name=f"rc{b}")
                for ch in range(NCH):
                    cs = slice(ch * 512, (ch + 1) * 512)
                    psd = pssm.tile([1, 512], F32, tag="sm", name=f"d{b}_{ch}")
                    for jt in range(NT):
                        nc.tensor.matmul(psd[:], ones_sb[:], e_sb[:, jt, cs],
                                         start=(jt == 0), stop=(jt == NT - 1))
                    nc.vector.tensor_copy(drow[:, cs], psd[:])
                for ch in range(NCH):
                    cs = slice(ch * 512, (ch + 1) * 512)
                    bc = pssm.tile([128, 512], F32, tag="sm", name=f"bc{b}_{ch}")
                    nc.tensor.matmul(bc[:], onescol_sb[:], drow[:, cs],
                                     start=True, stop=True)
                    nc.vector.reciprocal_approx_fast(recip_sb[:, cs], bc[:])
                return recip_sb

            def pv(b, vT_sb, e_sb, recip_sb):
                ou_sb = oup.tile([128, CT, N], F8, tag="ou", name=f"ou{b}")
                for ct in range(CT):
                    ps = psq.tile([128, N], F32, tag="mm", name=f"pso{b}_{ct}")
                    for jtp in range(4):
                        lhs = vT_sb[:, 2 * jtp:2 * jtp + 2, ct * 128:(ct + 1) * 128]
                        for ch in range(NCH):
                            cs = slice(ch * 512, (ch + 1) * 512)
                            nc.tensor.matmul(ps[:, cs], lhs,
                                             e_sb[:, 2 * jtp:2 * jtp + 2, cs],
                                             start=(jtp == 0), stop=(jtp == 3),
                                             perf_mode=DR)
                    nc.vector.tensor_tensor(ou_sb[:, ct, :], ps[:],
                                            recip_sb[:], Alu.mult)
                return ou_sb

            out_engs = [nc.sync, nc.scalar, nc.gpsimd, nc.sync]

            def proj_tile(b, ou_sb, xts, ot, on_act):
                ps = psq.tile([128, N], F32, tag="mm", name=f"psp{b}_{ot}")
                for ctp in range(2):
                    lhs = w_sb["wp"][:, 2 * ctp:2 * ctp + 2, ot * 128:(ot + 1) * 128]
                    for ch in range(NCH):
                        cs = slice(ch * 512, (ch + 1) * 512)
                        nc.tensor.matmul(ps[:, cs], lhs,
                                         ou_sb[:, 2 * ctp:2 * ctp + 2, cs],
                                         start=(ctp == 0), stop=(ctp == 1),
                                         perf_mode=DR)
                o_sb = outp.tile([128, N], F32, tag="o", name=f"o{b}_{ot}")
                bias = vec_sb["bpp"][:, ot:ot + 1]
                if on_act:
                    nc.scalar.activation(o_sb[:], ps[:], Act.Identity,
                                         bias=bias, scale=1.0 / (WS * OUS))
                else:
                    nc.vector.tensor_scalar(o_sb[:], ps[:], 1.0 / (WS * OUS),
                                            bias, Alu.mult, Alu.add)
                nc.vector.tensor_tensor(o_sb[:], o_sb[:], xts[ot][:], Alu.add)
                out_engs[ot].dma_start(out_ext[b, :, ot, :], o_sb[:])

            # ---- software pipeline over the two batch items ----
            h0 = gn_full(0, x0)

            # Q/K stretch for item 0 (ACT evictions) with item 1's GroupNorm
            # stats interleaved on the otherwise-idle DVE.
            q0 = qkp.tile([128, CT, N], F8, tag="q", name="q0")
            k0 = qkp.tile([128, CT, N], F8, tag="k", name="k0")
            mv1 = smallp.tile([128, CT, 2], F32, tag="mv", name="mv1")
            for t in range(CT):
                qk_tile(0, h0, q0, "wq", "bq", t)
                qk_tile(0, h0, k0, "wk", "bk", t)
                gn_stats(1, x1, mv1, t)
            h1 = gn_tail(1, x1, mv1)

            v0 = vp.tile([128, NT, 512], F8, tag="vT", name="vT0")
            for jt in range(NT):
                v_tile(0, h0, v0, jt)

            # item0 S-phase interleaved with item1 QKV
            e0 = ep.tile([128, NT, N], F8, tag="e", name="e0")
            q1 = qkp.tile([128, CT, N], F8, tag="q", name="q1")
            k1 = qkp.tile([128, CT, N], F8, tag="k", name="k1")
            v1 = vp.tile([128, NT, 512], F8, tag="vT", name="vT1")
            for r in range(CT):
                s_tile(0, q0, k0, e0, 2 * r)
                s_tile(0, q0, k0, e0, 2 * r + 1)
                qk_tile(1, h1, q1, "wq", "bq", r)
                qk_tile(1, h1, k1, "wk", "bk", r)
                v_tile(1, h1, v1, 2 * r)
                v_tile(1, h1, v1, 2 * r + 1)

            r0 = dsum(0, e0)
            ou0 = pv(0, v0, e0, r0)

            # item0 proj interleaved with item1 S-phase
            e1 = ep.tile([128, NT, N], F8, tag="e", name="e1")
            for r in range(CT):
                proj_tile(0, ou0, x0, r, on_act=(r < 2))
                s_tile(1, q1, k1, e1, 2 * r)
                s_tile(1, q1, k1, e1, 2 * r + 1)

            r1 = dsum(1, e1)
            ou1 = pv(1, v1, e1, r1)
            for r in range(CT):
                proj_tile(1, ou1, x1, r, on_act=True)

    nc.compile()
    return nc


def _prep_vec(v):
    # [C] f32 -> [128, CT] with v_sb[p, t] = v[t*128 + p]
    return np.ascontiguousarray(
        np.asarray(v, dtype=np.float32).reshape(CT, 128).T)


def _prep_w(w):
    # [C, C] (out, in) -> lhsT layout [128, CT, 512] fp8e4, pre-scaled by WS:
    # w_sb[p, it, o] = w[o, it*128 + p] * WS
    wT = np.asarray(w, dtype=np.float32).T * WS
    arr = wT.reshape(CT, 128, C).transpose(1, 0, 2)
    return np.clip(np.ascontiguousarray(arr), -240.0, 240.0).astype(
        ml_dtypes.float8_e4m3)


def kernel(x, gamma, beta, wq, bq, wk, bk, wv, bv, wp, bp):
    from concourse.bass_utils import run_bass_kernel_spmd

    nc = _CACHE.get("nc")
    if nc is None:
        nc = _CACHE["nc"] = _build_bass()

    x = np.asarray(x, dtype=np.float32)
    # [16, C, H, W] -> [16, 128, CT, N] bf16
    xr = np.ascontiguousarray(
        x.reshape(B_TOT, CT, 128, N).transpose(0, 2, 1, 3)).astype(
        ml_dtypes.bfloat16)

    bpp = np.asarray(wp, np.float32) @ np.asarray(bv, np.float32) \
        + np.asarray(bp, np.float32)
    sel = np.kron(np.eye(128 // GS, dtype=np.float32),
                  np.full((GS, GS), 1.0 / GS, dtype=np.float32))
    cb = np.empty((128, CB_W), dtype=np.float32)
    for i, v in enumerate((gamma, beta, bq, bk, bpp)):
        cb[:, i * CT:(i + 1) * CT] = _prep_vec(v)
    cb[:, NVEC * CT:] = sel
    common = {
        "wq": _prep_w(wq), "wk": _prep_w(wk), "wv": _prep_w(wv),
        "wp": _prep_w(wp), "cb": cb,
    }
    in_maps = [
        {"x": np.ascontiguousarray(xr[c * BPC:(c + 1) * BPC]), **common}
        for c in range(NCORES)
    ]
    res = run_bass_kernel_spmd(nc, in_maps, core_ids=list(range(NCORES)))
    # [BPC, 128, CT, N] per core -> [16, C, H, W]
    out = np.concatenate([r["out"] for r in res.results], axis=0)
    return np.ascontiguousarray(
        out.transpose(0, 2, 1, 3)).reshape(B_TOT, C, H, W)
